# revision 11
# baseline (speedup 1.0000x reference)
"""NVFP4-fake-quant MLP (x@w1.T -> gelu -> @w2.T) on 8 trn2 NeuronCores.

Sharding (megatron tensor-parallel on the hidden dim):
  core c holds w1 rows [c*2048:(c+1)*2048], w2 cols [c*2048:(c+1)*2048],
  and x rows [c*1024:(c+1)*1024] (for distributed x-quantization).

Exact quantization (v2 — single-region magic rounds):
  e4m3 block scales via exponent-mask + magic RNE (as before); e2m1
  rounding now uses the SAME exponent-mask trick on the signed value
  (grid = 2^floor(log2|r|)/2, clamped to [1/2, 2]); magic-add RNE works
  for signed r directly, so no abs/sign ops and no 3-region split.
  ~8 wide vector ops per element vs 16 in v1.

Schedule (v2):
  w1-amax scan is the first DMA priority; AR1 fires ~150us in; w1 quant +
  SBUF-direct w1T transposes chase it so phase 1 starts ~0.5ms (vs 1.7ms).
  xq AllGather is split in 4 column chunks so it fully overlaps.  w2
  amax/AR2/quant/pre-transpose all ride inside phase 1 (quant partly on
  gpsimd to keep DVE under the PE roofline).  w2 is pre-transposed to
  DRAM so the phase boundary is a straight 16MB load.  Phase-2 partials
  are bf16 (halves parts+ReduceScatter traffic); RS chunks overlap
  phase-2 compute and outputs are converted bf16->f32 per chunk.
"""
import os
import sys
import numpy as np

if "/opt/trn_rl_repo" not in sys.path:
    sys.path.insert(0, "/opt/trn_rl_repo")

f32 = np.float32

B, D_IN, HID, D_OUT = 8192, 4096, 16384, 4096
NCORES = 8
BSH = B // NCORES          # 1024 x-rows quantized per core
HSH = HID // NCORES        # 2048 hidden units per core
NBT = B // 128             # 64 b-tiles
NK1 = D_IN // 128          # 32 k-tiles, first matmul
NK2 = HSH // 128           # 16 k-tiles, second matmul
RSCH = 8                   # reduce-scatter chunks
RSROWS = B // RSCH         # 1024 rows per RS chunk
NJX = 4                    # xq AllGather column chunks (256 b-rows each)
JW = BSH // NJX            # 256

C_HALF = float(f32(1.5 * 2 ** 22))       # e2m1 magic: grid = s/2, scaled by s
E4M3_MAGIC = float(f32(1.5 * 2 ** 20))   # e4m3 magic: grid = s*2^-3
EXPMASK = 0x7F800000

_BUILT = {}


def _q_chain(nc, mybir, pf, pn, src, out, c1, ts, W, g_eng):
    """Quantize src [128, W] f32 -> out [128, W] bf16 (= e2m1 * e4m3 bscale).

    c1: 1/(6*tensor_scale) (imm or [128,1] AP); ts: tensor_scale.
    g_eng: engine for the offloadable wide ops (nc.vector or nc.gpsimd).
    """
    OP = mybir.AluOpType
    U32 = mybir.dt.uint32
    FP32 = mybir.dt.float32
    BF16 = mybir.dt.bfloat16
    NB = W // 16
    v = nc.vector

    amax = pn.tile([128, NB], FP32, tag="n_amax", name="n_amax")
    v.tensor_reduce(amax[:], src.rearrange("p (nb b) -> p nb b", b=16),
                    axis=mybir.AxisListType.X, op=OP.max,
                    apply_absolute_value=True)
    vq = pn.tile([128, NB], FP32, tag="n_vq", name="n_vq")
    v.tensor_scalar(vq[:], amax[:], c1, None, OP.mult)
    cbn = pn.tile([128, NB], FP32, tag="n_cb", name="n_cb")
    v.tensor_scalar(cbn[:].bitcast(U32), vq[:].bitcast(U32),
                    EXPMASK, None, OP.bitwise_and)
    v.tensor_scalar(cbn[:], cbn[:], E4M3_MAGIC, None, OP.mult)
    bs = pn.tile([128, NB], FP32, tag="n_bs", name="n_bs")
    v.tensor_tensor(bs[:], vq[:], cbn[:], OP.add)
    v.tensor_tensor(bs[:], bs[:], cbn[:], OP.subtract)
    bs16 = pn.tile([128, NB], BF16, tag="n_bs16", name="n_bs16")
    v.tensor_scalar(bs16[:], bs[:], 2.0 ** -6, None, OP.max)
    eff = pn.tile([128, NB], FP32, tag="n_eff", name="n_eff")
    v.tensor_scalar(eff[:], bs[:], 2.0 ** -6, ts, OP.max, OP.mult)
    rec = pn.tile([128, NB], FP32, tag="n_rec", name="n_rec")
    v.reciprocal(rec[:], eff[:])

    a = pf.tile([128, W], FP32, tag="q_a", name="q_a")
    bt = pf.tile([128, W], FP32, tag="q_b", name="q_b")
    # r = src / eff  (signed)
    v.tensor_tensor(a[:].rearrange("p (nb b) -> p nb b", b=16),
                    src.rearrange("p (nb b) -> p nb b", b=16),
                    rec[:, :, None].to_broadcast([128, NB, 16]), OP.mult)
    # clamp to [-6, 6]
    g_eng.tensor_scalar(a[:], a[:], -6.0, 6.0, OP.max, OP.min)
    # s0 = exponent mask of |r|; cb = max(s0,1) * C_HALF
    v.tensor_scalar(bt[:].bitcast(U32), a[:].bitcast(U32),
                    EXPMASK, None, OP.bitwise_and)
    v.tensor_scalar(bt[:], bt[:], 1.0, C_HALF, OP.max, OP.mult)
    # magic RNE at grid s/2 (signed): q = (r + cb) - cb
    g_eng.tensor_tensor(a[:], a[:], bt[:], OP.add)
    v.tensor_tensor(bt[:], a[:], bt[:], OP.subtract)
    # out = q * bscale  (exact in bf16)
    v.tensor_tensor(out.rearrange("p (nb b) -> p nb b", b=16),
                    bt[:].rearrange("p (nb b) -> p nb b", b=16),
                    bs16[:, :, None].to_broadcast([128, NB, 16]), OP.mult)


def _build(isc, hsc):
    from contextlib import ExitStack
    import concourse.bass as bass
    import concourse.tile as tile
    from concourse import bacc, mybir

    OP = mybir.AluOpType
    AF = mybir.ActivationFunctionType
    FP32 = mybir.dt.float32
    BF16 = mybir.dt.bfloat16

    c1x = float(f32(1.0) / (f32(6.0) * f32(isc)))
    c1h = float(f32(1.0) / (f32(6.0) * f32(hsc)))
    inv2688 = float(f32(1.0) / f32(2688.0))
    RG = [list(range(NCORES))]

    nc = bacc.Bacc("TRN2", target_bir_lowering=False, debug=False,
                   num_devices=NCORES)
    x_sh = nc.dram_tensor("x_sh", [BSH, D_IN], FP32, kind="ExternalInput").ap()
    w1_sh = nc.dram_tensor("w1_sh", [HSH, D_IN], FP32, kind="ExternalInput").ap()
    w2_sh = nc.dram_tensor("w2_sh", [D_OUT, HSH], FP32, kind="ExternalInput").ap()
    out_sh = nc.dram_tensor("out_sh", [BSH, D_OUT], FP32,
                            kind="ExternalOutput").ap()

    with tile.TileContext(nc) as tc, ExitStack() as top:
        dram = top.enter_context(tc.tile_pool(name="dram", bufs=1, space="DRAM"))
        amax_stage = dram.tile([128, 2], FP32, tag="amax_stage", name="amax_stage")
        s1loc = dram.tile([1, 1], FP32, tag="s1loc", name="s1loc")
        s2loc = dram.tile([1, 1], FP32, tag="s2loc", name="s2loc")
        s1sh = dram.tile([1, 1], FP32, tag="s1sh", name="s1sh", addr_space="Shared")
        s2sh = dram.tile([1, 1], FP32, tag="s2sh", name="s2sh", addr_space="Shared")
        xq_loc = dram.tile([BSH, D_IN], BF16, tag="xq_loc", name="xq_loc")
        xqT_locs = [dram.tile([D_IN, JW], BF16, tag=f"xqT_loc{j}",
                              name=f"xqT_loc{j}") for j in range(NJX)]
        xqT_fulls = [dram.tile([NCORES * D_IN, JW], BF16, tag=f"xqT_full{j}",
                               name=f"xqT_full{j}", addr_space="Shared")
                     for j in range(NJX)]
        w1q = dram.tile([HSH, D_IN], BF16, tag="w1q", name="w1q")
        w2q = dram.tile([D_OUT, HSH], BF16, tag="w2q", name="w2q")
        w2t_dram = dram.tile([HSH, D_OUT], BF16, tag="w2t_dram", name="w2t_dram")
        hq = dram.tile([B, HSH], BF16, tag="hq", name="hq")
        parts = [dram.tile([RSROWS, D_OUT], FP32, name=f"part{c}", tag=f"part{c}")
                 for c in range(RSCH)]
        rsouts = [dram.tile([128, D_OUT], FP32, name=f"rsout{c}", tag=f"rsout{c}")
                  for c in range(RSCH)]

        singles = top.enter_context(tc.tile_pool(name="singles", bufs=1))

        # w1T pre-reserved so quant-chasing transposes land directly in SBUF.
        w1T_cm = tc.tile_pool(name="w1T", bufs=1)
        w1T_pool = w1T_cm.__enter__()
        w1T = w1T_pool.tile([128, NK1, HSH], BF16, tag="w1T", name="w1T")

        # ================= Phase 0 =================
        with tc.tile_pool(name="p0src", bufs=2) as p0src, \
             tc.tile_pool(name="pf", bufs=2) as pf, \
             tc.tile_pool(name="pn", bufs=2) as pn, \
             tc.tile_pool(name="pb", bufs=2) as pb:
            # ---- w1 amax scan (top DMA priority) -> AllReduce(max) ----
            acc1 = singles.tile([128, 1], FP32, tag="acc1", name="acc1")
            for i in range(HSH // 128):
                wt = p0src.tile([128, 2048], FP32, tag="p0ld", name="wamax")
                nc.scalar.dma_start(wt[:], w1_sh[i * 128:(i + 1) * 128, 0:2048])
                wt2 = p0src.tile([128, 2048], FP32, tag="p0ld", name="wamax2")
                nc.sync.dma_start(wt2[:], w1_sh[i * 128:(i + 1) * 128, 2048:4096])
                am = pn.tile([128, 1], FP32, tag="am_w", name="am_w")
                nc.vector.tensor_reduce(am[:], wt[:], axis=mybir.AxisListType.X,
                                        op=OP.max, apply_absolute_value=True)
                am2 = pn.tile([128, 1], FP32, tag="am_w2", name="am_w2")
                nc.vector.tensor_reduce(am2[:], wt2[:], axis=mybir.AxisListType.X,
                                        op=OP.max, apply_absolute_value=True)
                nc.vector.tensor_tensor(am[:], am[:], am2[:], OP.max)
                if i == 0:
                    nc.vector.tensor_copy(acc1[:], am[:])
                else:
                    nc.vector.tensor_tensor(acc1[:], acc1[:], am[:], OP.max)
            nc.sync.dma_start(amax_stage[:, 0:1], acc1[:])
            rowv1 = singles.tile([1, 128], FP32, tag="rowv1", name="rowv1")
            nc.sync.dma_start(
                rowv1[:], amax_stage[:, 0:1].rearrange("p c -> (p c)").unsqueeze(0))
            red1 = singles.tile([1, 1], FP32, tag="red1", name="red1")
            nc.vector.tensor_reduce(red1[:], rowv1[:],
                                    axis=mybir.AxisListType.X, op=OP.max)
            nc.sync.dma_start(s1loc[:], red1[:])
            nc.gpsimd.collective_compute(
                "AllReduce", OP.max, replica_groups=RG,
                ins=[s1loc[:].opt()], outs=[s1sh[:].opt()])
            sam1 = singles.tile([128, 1], FP32, tag="sam1", name="sam1")
            ap1 = s1sh[:]
            nc.gpsimd.dma_start(sam1[:], bass.AP(
                tensor=ap1.tensor, offset=ap1.offset,
                ap=[[0, 128]] + list(ap1.ap)[1:]))
            tsw1 = singles.tile([128, 1], FP32, tag="tsw1", name="tsw1")
            nc.vector.tensor_scalar(tsw1[:], sam1[:], inv2688, None, OP.mult)
            dw1 = singles.tile([128, 1], FP32, tag="dw1", name="dw1")
            nc.vector.tensor_scalar(dw1[:], tsw1[:], 6.0, None, OP.mult)
            rdw1 = singles.tile([128, 1], FP32, tag="rdw1", name="rdw1")
            nc.vector.reciprocal(rdw1[:], dw1[:])
            s_h = singles.tile([128, 1], FP32, tag="s_h", name="s_h")
            nc.vector.tensor_scalar(s_h[:], tsw1[:], float(isc), None, OP.mult)

            # ---- x quantize (sync-ring loads; overlaps the w1 scan) ----
            for i in range(BSH // 128):
                for c in range(2):
                    xt = p0src.tile([128, 2048], FP32, tag="p0ldx", name="xt")
                    nc.sync.dma_start(
                        xt[:], x_sh[i * 128:(i + 1) * 128,
                                    c * 2048:(c + 1) * 2048])
                    for h in range(2):
                        xo = pb.tile([128, 1024], BF16, tag="q_out", name="xo")
                        _q_chain(nc, mybir, pf, pn, xt[:, h * 1024:(h + 1) * 1024],
                                 xo[:], c1x, float(isc), 1024, nc.vector)
                        nc.sync.dma_start(
                            xq_loc[i * 128:(i + 1) * 128,
                                   c * 2048 + h * 1024:c * 2048 + (h + 1) * 1024],
                            xo[:])
            # xq -> xqT transposes per column chunk, then chunked AllGathers
            for j in range(NJX):
                for k in range(NK1):
                    xtt = p0src.tile([128, JW], BF16, tag="xtt", name="xtt")
                    nc.sync.dma_start(
                        xtt[:], xq_loc[j * JW:(j + 1) * JW,
                                       k * 128:(k + 1) * 128], transpose=True)
                    nc.sync.dma_start(xqT_locs[j][k * 128:(k + 1) * 128, :],
                                      xtt[:])
                nc.gpsimd.collective_compute(
                    "AllGather", OP.bypass, replica_groups=RG,
                    ins=[xqT_locs[j][:].opt()], outs=[xqT_fulls[j][:].opt()])

            # ---- w1 quantize (after AR1), group-ordered; transposes chase ----
            for g in range(4):
                for rt in range(4):
                    i = g * 4 + rt
                    for c in range(2):
                        wt = p0src.tile([128, 2048], FP32, tag="p0ld", name="wqt")
                        nc.scalar.dma_start(
                            wt[:], w1_sh[i * 128:(i + 1) * 128,
                                         c * 2048:(c + 1) * 2048])
                        for h in range(2):
                            wo = pb.tile([128, 1024], BF16, tag="q_out", name="wo")
                            _q_chain(nc, mybir, pf, pn,
                                     wt[:, h * 1024:(h + 1) * 1024], wo[:],
                                     rdw1[:], tsw1[:], 1024, nc.vector)
                            nc.sync.dma_start(
                                w1q[i * 128:(i + 1) * 128,
                                    c * 2048 + h * 1024:c * 2048 + (h + 1) * 1024],
                                wo[:])
                # SBUF-direct wide transposes for this 512-row group
                for k in range(NK1):
                    nc.sync.dma_start(
                        w1T[:, k, g * 512:(g + 1) * 512],
                        w1q[g * 512:(g + 1) * 512, k * 128:(k + 1) * 128],
                        transpose=True)

        # ================= Phase 1 =================
        with tc.tile_pool(name="xb", bufs=2) as xb_pool, \
             tc.tile_pool(name="p1src", bufs=2) as p1src, \
             tc.tile_pool(name="pg", bufs=2) as pg, \
             tc.tile_pool(name="pf1", bufs=2) as pf1, \
             tc.tile_pool(name="pn1", bufs=2) as pn1, \
             tc.tile_pool(name="pb1", bufs=2) as pb1, \
             tc.tile_pool(name="ps1", bufs=8, space="PSUM") as ps1:
            acc2 = singles.tile([128, 1], FP32, tag="acc2", name="acc2")
            tsw2 = singles.tile([128, 1], FP32, tag="tsw2", name="tsw2")
            rdw2 = singles.tile([128, 1], FP32, tag="rdw2", name="rdw2")
            s_o = singles.tile([128, 1], FP32, tag="s_o", name="s_o")

            for t in range(NBT):
                ci, off = t // 8, (t % 8) * 128
                j, col = (t % 8) // 2, (t % 2) * 128
                g0 = ci * BSH + off
                xb = xb_pool.tile([128, NK1, 128], BF16, tag="xb", name="xb")
                nc.sync.dma_start(
                    xb[:],
                    xqT_fulls[j][ci * D_IN:(ci + 1) * D_IN, col:col + 128]
                    .rearrange("(k p) c -> p k c", p=128))
                pss = [ps1.tile([128, 512], FP32, name="ps", tag="ps")
                       for _ in range(4)]
                for k in range(NK1):
                    for n in range(4):
                        nc.tensor.matmul(
                            pss[n][:], lhsT=xb[:, k, :],
                            rhs=w1T[:, k, n * 512:(n + 1) * 512],
                            start=(k == 0), stop=(k == NK1 - 1))
                g = pg.tile([128, 2048], FP32, tag="q_g", name="q_g")
                for n in range(4):
                    nc.scalar.activation(g[:, n * 512:(n + 1) * 512], pss[n][:],
                                         AF.Gelu, scale=s_h[:])
                geng = nc.gpsimd if t >= 16 else nc.vector
                for h in range(2):
                    ho = pb1.tile([128, 1024], BF16, tag="q_out", name="ho")
                    _q_chain(nc, mybir, pf1, pn1, g[:, h * 1024:(h + 1) * 1024],
                             ho[:], c1h, float(hsc), 1024, geng)
                    nc.sync.dma_start(
                        hq[g0:g0 + 128, h * 1024:(h + 1) * 1024], ho[:])

                if t < 8:
                    # w2 amax scan: 4 row-tiles per b-tile
                    for u in range(4):
                        i2 = 4 * t + u
                        wt3 = p1src.tile([128, 2048], FP32, tag="p1ld", name="wt3")
                        nc.scalar.dma_start(wt3[:], w2_sh[i2 * 128:(i2 + 1) * 128, :])
                        am2 = pn1.tile([128, 1], FP32, tag="am_w2", name="am_w2")
                        nc.vector.tensor_reduce(am2[:], wt3[:],
                                                axis=mybir.AxisListType.X,
                                                op=OP.max,
                                                apply_absolute_value=True)
                        if i2 == 0:
                            nc.vector.tensor_copy(acc2[:], am2[:])
                        else:
                            nc.vector.tensor_tensor(acc2[:], acc2[:], am2[:],
                                                    OP.max)
                elif t == 8:
                    nc.sync.dma_start(amax_stage[:, 1:2], acc2[:])
                    rowv2 = singles.tile([1, 128], FP32, tag="rowv2",
                                         name="rowv2")
                    nc.sync.dma_start(
                        rowv2[:],
                        amax_stage[:, 1:2].rearrange("p c -> (p c)").unsqueeze(0))
                    red2 = singles.tile([1, 1], FP32, tag="red2", name="red2")
                    nc.vector.tensor_reduce(red2[:], rowv2[:],
                                            axis=mybir.AxisListType.X, op=OP.max)
                    nc.sync.dma_start(s2loc[:], red2[:])
                    nc.gpsimd.collective_compute(
                        "AllReduce", OP.max, replica_groups=RG,
                        ins=[s2loc[:].opt()], outs=[s2sh[:].opt()])
                    sam2 = singles.tile([128, 1], FP32, tag="sam2", name="sam2")
                    ap2 = s2sh[:]
                    nc.gpsimd.dma_start(sam2[:], bass.AP(
                        tensor=ap2.tensor, offset=ap2.offset,
                        ap=[[0, 128]] + list(ap2.ap)[1:]))
                    nc.vector.tensor_scalar(tsw2[:], sam2[:], inv2688, None,
                                            OP.mult)
                    dw2 = singles.tile([128, 1], FP32, tag="dw2", name="dw2")
                    nc.vector.tensor_scalar(dw2[:], tsw2[:], 6.0, None, OP.mult)
                    nc.vector.reciprocal(rdw2[:], dw2[:])
                    nc.vector.tensor_scalar(s_o[:], tsw2[:], float(hsc), None,
                                            OP.mult)
                elif 16 <= t < 48:
                    # w2 quantize: 1 row-tile per b-tile
                    wi = t - 16
                    wt2 = p1src.tile([128, 2048], FP32, tag="p1ld", name="wt2")
                    nc.scalar.dma_start(wt2[:], w2_sh[wi * 128:(wi + 1) * 128, :])
                    for h in range(2):
                        wo2 = pb1.tile([128, 1024], BF16, tag="q_out", name="wo2")
                        _q_chain(nc, mybir, pf1, pn1,
                                 wt2[:, h * 1024:(h + 1) * 1024], wo2[:],
                                 rdw2[:], tsw2[:], 1024, nc.gpsimd)
                        nc.sync.dma_start(
                            w2q[wi * 128:(wi + 1) * 128,
                                h * 1024:(h + 1) * 1024], wo2[:])


        # ================= Phase 2 =================
        w1T_cm.__exit__(None, None, None)
        with tc.tile_pool(name="w2T", bufs=1) as w2T_pool, \
             tc.tile_pool(name="hT", bufs=3) as hT_pool, \
             tc.tile_pool(name="osb", bufs=4) as osb, \
             tc.tile_pool(name="ps2", bufs=8, space="PSUM") as ps2:
            w2T = w2T_pool.tile([128, NK2, D_OUT], BF16, tag="w2T", name="w2T")
            for hf in range(2):
                for k in range(NK2):
                    nc.sync.dma_start(
                        w2T[:, k, hf * 2048:(hf + 1) * 2048],
                        w2q[hf * 2048:(hf + 1) * 2048,
                            k * 128:(k + 1) * 128], transpose=True)
            for sb in range(B // 256):
                r0 = sb * 256
                hT = hT_pool.tile([128, NK2, 256], BF16, tag="hT", name="hT")
                for k in range(NK2):
                    nc.sync.dma_start(hT[:, k, :],
                                      hq[r0:r0 + 256, k * 128:(k + 1) * 128],
                                      transpose=True)
                for b in range(2):
                    row = r0 + b * 128
                    c = row // RSROWS
                    crow = row % RSROWS
                    for hf in range(2):
                        pss = [ps2.tile([128, 512], FP32, name="ps2", tag="ps2")
                               for _ in range(4)]
                        for k in range(NK2):
                            for n in range(4):
                                nc.tensor.matmul(
                                    pss[n][:],
                                    lhsT=hT[:, k, b * 128:(b + 1) * 128],
                                    rhs=w2T[:, k,
                                            hf * 2048 + n * 512:
                                            hf * 2048 + (n + 1) * 512],
                                    start=(k == 0), stop=(k == NK2 - 1))
                        ot = osb.tile([128, 2048], FP32, tag="ot", name="ot")
                        for n in range(4):
                            nc.scalar.activation(ot[:, n * 512:(n + 1) * 512],
                                                 pss[n][:], AF.Copy,
                                                 scale=s_o[:])
                        nc.sync.dma_start(
                            parts[c][crow:crow + 128,
                                     hf * 2048:(hf + 1) * 2048], ot[:])
                if sb % 4 == 3:
                    c = sb // 4
                    nc.gpsimd.collective_compute(
                        "ReduceScatter", OP.add, replica_groups=RG,
                        ins=[parts[c][:].opt()], outs=[rsouts[c][:].opt()])
                    nc.sync.dma_start(out_sh[c * 128:(c + 1) * 128, :],
                                      rsouts[c][:])
    nc.compile()
    return nc


def _get_built(isc, hsc):
    key = (float(isc), float(hsc))
    if key not in _BUILT:
        _BUILT[key] = _build(float(isc), float(hsc))
    return _BUILT[key]


def run(x, w1, w2, input_scale, hidden_scale, trace=False):
    from concourse import bass_utils
    isc = float(np.asarray(input_scale).reshape(-1)[0])
    hsc = float(np.asarray(hidden_scale).reshape(-1)[0])
    nc = _get_built(isc, hsc)
    x = np.ascontiguousarray(x, dtype=np.float32)
    w1 = np.ascontiguousarray(w1, dtype=np.float32)
    w2 = np.ascontiguousarray(w2, dtype=np.float32)
    in_maps = []
    for c in range(NCORES):
        in_maps.append({
            "x_sh": x[c * BSH:(c + 1) * BSH, :],
            "w1_sh": np.ascontiguousarray(w1[c * HSH:(c + 1) * HSH, :]),
            "w2_sh": np.ascontiguousarray(w2[:, c * HSH:(c + 1) * HSH]),
        })
    res = bass_utils.run_bass_kernel_spmd(
        nc, in_maps, core_ids=list(range(NCORES)), trace=trace)
    out = np.empty((B, D_OUT), dtype=np.float32)
    for r in range(NCORES):
        o = res.results[r]["out_sh"]
        for c in range(RSCH):
            out[c * RSROWS + r * 128:c * RSROWS + (r + 1) * 128, :] = \
                o[c * 128:(c + 1) * 128, :]
    return out, res


def kernel(x, w1, w2, input_scale, hidden_scale):
    out, _ = run(x, w1, w2, input_scale, hidden_scale, trace=False)
    return out
